# revision 5
# baseline (speedup 1.0000x reference)
"""Causal self-attention (T=4096, C=2048, 16 heads) on 8 TRN2 NeuronCores.

Sharding: tensor-parallel over heads (2 heads/core) for QKV + attention,
then per-head AllToAlls redistribute the attention output to
token-parallel (512 tokens/core) for the output projection; each core
computes full output rows for its token slice and the host concatenates.

All matmuls run bf16 (host converts, halving DMA bytes; PSUM stays
fp32). Scores are computed transposed (keys on partitions, queries
free) so P@V needs no transposes; causal masking is a bf16 multiply
with precomputed diagonal masks and upper-triangle blocks are skipped.

Performance structure:
- Every bulk HBM tensor is PRE-TILED on the host into the exact SBUF
  layout, so each load is one dma_start with 8-32KB descriptors (the
  DMA rings charge ~155ns per descriptor regardless of size, so 1KB
  descriptors cap loads at ~105GB/s while 16KB descriptors saturate).
- Exp runs on [128,1024] pairs of score tiles; on the last (diagonal)
  pair of each chunk only the causally-live [128,2,256] sub-block is
  computed.
- Softmax denominators: DVE accumulates exp sums into a [128,1024]
  running tile, folds once per chunk, and a single ones-vector matmul
  per chunk reduces over partitions (per-k-tile ones-matmuls would
  burn ~60us of tensor time).
- Normalization happens on the SOURCE side of the AllToAll (reciprocal
  + partition-broadcast + DVE multiply, off the tensor path), so phase
  3 is pure DMA + matmul. The gpsimd partition-broadcast library is
  warmed up at t=0: the first gpsimd op of a new type triggers a ~14us
  LIBRARY_RELOAD which otherwise lands mid-pipeline.
- qT/kT PSUM->SBUF copies run on the scalar engine (idle in phase 1)
  to keep the DVE free for phase-2 exp-sum accumulation.
"""
import sys
import types

sys.path.insert(0, "/opt/trn_rl_repo")

import ml_dtypes
import numpy as np

from concourse import bacc, tile
import concourse.mybir as mybir
from concourse.bass_utils import run_bass_kernel_spmd

F32 = mybir.dt.float32
BF16 = mybir.dt.bfloat16
NP_BF16 = np.dtype(ml_dtypes.bfloat16)

T, C = 4096, 2048
H, D = 16, 128
W = 8                  # cores
HL = H // W            # heads per core (2)
CL = HL * D            # local attention-output columns (256)
KT = C // 128          # contraction tiles (16)
TC = 512               # token chunk
NC = T // TC           # 8
TL = T // W            # tokens per core for the projection (512)
SCALE = float(1.0 / np.sqrt(D))

TRACE = False          # test harness sets kernel.TRACE = True for profiling
LAST_RESULT = {}       # test harness reads exec_time_ns from here

_cache = {}


def _build():
    nc = bacc.Bacc("TRN2", target_bir_lowering=False, debug=False, num_devices=W)
    # host pre-tiles everything into [partition, free] SBUF layout
    xTt_d = nc.dram_tensor("xTt", [NC * 128, KT * TC], BF16,
                           kind="ExternalInput")
    wqkTt_d = nc.dram_tensor("wqkTt", [128, KT * 512], BF16,
                             kind="ExternalInput")
    wvTt_d = nc.dram_tensor("wvTt", [128, KT * CL], BF16,
                            kind="ExternalInput")
    wpTt_d = nc.dram_tensor("wpTt", [128, KT * C], BF16,
                            kind="ExternalInput")
    out_d = nc.dram_tensor("out", [TL, C], F32, kind="ExternalOutput")

    with tile.TileContext(nc) as tc:
        with tc.tile_pool(name="res", bufs=1) as res, \
             tc.tile_pool(name="dram", bufs=1, space="DRAM") as dram:
            # per-head A2A buffers (bf16): shard j = my token chunk j.
            a2a_in = [dram.tile([W, 128, TC], BF16, tag=f"a2a_in{h}",
                                name=f"a2a_in{h}") for h in range(HL)]
            a2a_out = [dram.tile([W, 128, TC], BF16, tag=f"a2a_out{h}",
                                 name=f"a2a_out{h}") for h in range(HL)]

            # resident q/k (transposed, [d, t]) and V ([s, d]), all bf16
            qT = [res.tile([128, T], BF16, tag=f"qT{h}", name=f"qT{h}")
                  for h in range(HL)]
            kT = [res.tile([128, T], BF16, tag=f"kT{h}", name=f"kT{h}")
                  for h in range(HL)]
            V = [res.tile([128, CL], BF16, tag=f"V{i}", name=f"V{i}")
                 for i in range(T // 128)]

            ones32 = res.tile([128, 1], F32, tag="ones32")
            nc.gpsimd.memset(ones32[:], 1.0)
            ones = res.tile([128, 1], BF16, tag="ones")
            nc.vector.tensor_copy(ones[:], ones32[:])

            # paired diagonal causal masks (keep where t >= s within the
            # tile): [mask_dk0 | mask_dk0+1] as one [128,1024] tile
            masks = []
            for dk0 in (0, 2):
                m32 = res.tile([128, 1024], F32, tag=f"m32_{dk0}",
                               name=f"m32_{dk0}")
                nc.gpsimd.memset(m32[:], 1.0)
                mb = res.tile([128, 1024], BF16, tag=f"mask{dk0}",
                              name=f"mask{dk0}")
                nc.vector.tensor_copy(mb[:], m32[:])
                for half in range(2):
                    nc.gpsimd.affine_select(
                        out=mb[:, half * 512:(half + 1) * 512],
                        in_=mb[:, half * 512:(half + 1) * 512],
                        compare_op=mybir.AluOpType.is_ge,
                        fill=0.0,
                        base=-128 * (dk0 + half),
                        channel_multiplier=-1,
                        pattern=[[1, 512]],
                    )
                masks.append(mb)

            # warm up the gpsimd partition-broadcast DSP library now, while
            # the initial DMAs stream: the first op of a type triggers a
            # ~14us LIBRARY_RELOAD that must not land mid-attention
            warm = res.tile([128, 1], F32, tag="warm")
            nc.gpsimd.partition_broadcast(warm[:], ones32[0:1, :])

            # ---------------- phase 1: QKV projection (bf16) ----------------
            with tc.tile_pool(name="wpool", bufs=1) as wpool, \
                 tc.tile_pool(name="xpool", bufs=2) as xpool, \
                 tc.tile_pool(name="ps1", bufs=3, space="PSUM") as ps1:
                wqk = wpool.tile([128, KT * 512], BF16, tag="wqk", name="wqk")
                nc.sync.dma_start(wqk[:], wqkTt_d.ap())

                def load_x_chunk(j):
                    xt = xpool.tile([128, KT * TC], BF16, tag="x",
                                    name=f"x{j}")
                    nc.sync.dma_start(
                        xt[:], xTt_d.ap()[j * 128:(j + 1) * 128, :])
                    return xt

                xt0 = load_x_chunk(0)
                wv = wpool.tile([128, KT * CL], BF16, tag="wv", name="wv")
                nc.sync.dma_start(wv[:], wvTt_d.ap())

                for j in range(NC):
                    xt = xt0 if j == 0 else load_x_chunk(j)
                    # qT/kT for both heads: out[d, t] accumulated over c
                    for m in range(4):
                        pq = ps1.tile([128, TC], F32, tag="pqk")
                        for k in range(KT):
                            nc.tensor.matmul(
                                pq[:],
                                wqk[:, k * 512 + m * 128:
                                    k * 512 + (m + 1) * 128],
                                xt[:, k * TC:(k + 1) * TC],
                                start=(k == 0), stop=(k == KT - 1))
                        dest = qT[m] if m < HL else kT[m - HL]
                        nc.scalar.copy(dest[:, j * TC:(j + 1) * TC], pq[:])
                    # V: out[t, d] accumulated over c
                    for tt in range(TC // 128):
                        pv = ps1.tile([128, CL], F32, tag="pv")
                        for k in range(KT):
                            nc.tensor.matmul(
                                pv[:],
                                xt[:, k * TC + tt * 128:
                                   k * TC + (tt + 1) * 128],
                                wv[:, k * CL:(k + 1) * CL],
                                start=(k == 0), stop=(k == KT - 1))
                        nc.scalar.copy(V[j * (TC // 128) + tt][:], pv[:])

            # ---------------- phases 2+3 pools ----------------
            with tc.tile_pool(name="ph2", bufs=3) as p2, \
                 tc.tile_pool(name="a2s", bufs=3) as a2s, \
                 tc.tile_pool(name="p3a", bufs=1) as p3a, \
                 tc.tile_pool(name="p2n", bufs=2) as p2n, \
                 tc.tile_pool(name="p3w", bufs=1) as p3w, \
                 tc.tile_pool(name="p3o", bufs=2) as p3o:
                # prefetch the projection weight during phase 2 (one
                # dispatch, 32KB descriptors)
                wp = p3w.tile([128, KT * C], BF16, tag="wp", name="wp")
                nc.sync.dma_start(
                    wp[:].rearrange("p (h x) -> p h x", h=2),
                    wpTt_d.ap().rearrange("p (h x) -> p h x", h=2),
                )

                attn_all = [None] * HL

                # ---------------- phase 2: attention (bf16) ----------------
                with tc.tile_pool(name="ps2s", bufs=2, space="PSUM") as ps2s, \
                     tc.tile_pool(name="ps2o", bufs=2, space="PSUM") as ps2o, \
                     tc.tile_pool(name="ps2d", bufs=1, space="PSUM") as ps2d:
                    pending = None
                    for h in range(HL):
                        for j in range(NC):
                            npairs = (j + 1) * 2
                            po = ps2o.tile([128, TC], F32, tag="po")
                            esum = p2n.tile([128, 1024], BF16, tag="esum")
                            for p in range(npairs):
                                k0, k1 = 2 * p, 2 * p + 1
                                dk0 = k0 - 4 * j
                                # causally-live column offset for this pair
                                off = 256 if dk0 == 2 else 0
                                ps = ps2s.tile([128, 1024], F32, tag="ps")
                                nc.tensor.matmul(
                                    ps[:, off:512],
                                    kT[h][:, k0 * 128:(k0 + 1) * 128],
                                    qT[h][:, j * TC + off:(j + 1) * TC],
                                    start=True, stop=True)
                                nc.tensor.matmul(
                                    ps[:, 512 + off:1024],
                                    kT[h][:, k1 * 128:(k1 + 1) * 128],
                                    qT[h][:, j * TC + off:(j + 1) * TC],
                                    start=True, stop=True)
                                if p == 1 and pending is not None:
                                    # previous chunk's denominator + norm,
                                    # deferred behind a pair of scores
                                    pending()
                                    pending = None
                                e2 = p2.tile([128, 1024], BF16, tag="e")
                                if off:
                                    nc.scalar.activation(
                                        e2[:].rearrange(
                                            "p (g q) -> p g q",
                                            g=2)[:, :, off:],
                                        ps[:].rearrange(
                                            "p (g q) -> p g q",
                                            g=2)[:, :, off:],
                                        mybir.ActivationFunctionType.Exp,
                                        scale=SCALE)
                                else:
                                    nc.scalar.activation(
                                        e2[:], ps[:],
                                        mybir.ActivationFunctionType.Exp,
                                        scale=SCALE)
                                if dk0 >= 0:
                                    # diagonal pair: zero out s > t entries
                                    mk = masks[dk0 // 2]
                                    if off:
                                        sl = e2[:].rearrange(
                                            "p (g q) -> p g q",
                                            g=2)[:, :, off:]
                                        nc.vector.tensor_mul(
                                            sl, sl,
                                            mk[:].rearrange(
                                                "p (g q) -> p g q",
                                                g=2)[:, :, off:])
                                    else:
                                        nc.vector.tensor_mul(
                                            e2[:], e2[:], mk[:])
                                if p == 0:
                                    nc.vector.tensor_copy(esum[:], e2[:])
                                elif off:
                                    se = esum[:].rearrange(
                                        "p (g q) -> p g q", g=2)[:, :, off:]
                                    nc.vector.tensor_add(
                                        se, se,
                                        e2[:].rearrange(
                                            "p (g q) -> p g q",
                                            g=2)[:, :, off:])
                                else:
                                    nc.vector.tensor_add(
                                        esum[:], esum[:], e2[:])
                                nc.tensor.matmul(
                                    po[:, off:512],
                                    V[k0][:, h * 128:(h + 1) * 128],
                                    e2[:, off:512],
                                    start=(p == 0), stop=False)
                                nc.tensor.matmul(
                                    po[:, off:512],
                                    V[k1][:, h * 128:(h + 1) * 128],
                                    e2[:, 512 + off:1024],
                                    start=False, stop=(p == npairs - 1))
                            # fold the two esum halves (DVE, off tensor path)
                            nc.vector.tensor_add(
                                esum[:, 0:512], esum[:, 0:512],
                                esum[:, 512:1024])

                            def make_norm(h=h, j=j, po=po, esum=esum):
                                def norm():
                                    pd = ps2d.tile([1, TC], F32, tag="pd")
                                    nc.tensor.matmul(pd[:], ones[:],
                                                     esum[:, 0:512],
                                                     start=True, stop=True)
                                    rec = p2n.tile([1, TC], F32, tag="rec")
                                    nc.vector.reciprocal(rec[:], pd[:])
                                    r128 = p2n.tile([128, TC], F32,
                                                    tag="r128")
                                    nc.gpsimd.partition_broadcast(
                                        r128[:], rec[:])
                                    att = a2s.tile([128, TC], BF16,
                                                   tag="att")
                                    nc.vector.tensor_mul(att[:], po[:],
                                                         r128[:])
                                    nc.sync.dma_start(a2a_in[h][j, :, :],
                                                      att[:])
                                return norm

                            pending = make_norm()
                        # flush the last chunk's normalize, then fire this
                        # head's A2A; head 0's collective overlaps head 1
                        pending()
                        pending = None
                        nc.gpsimd.collective_compute(
                            "AllToAll",
                            mybir.AluOpType.bypass,
                            ins=[a2a_in[h].opt()],
                            outs=[a2a_out[h].opt()],
                            replica_groups=[list(range(W))],
                        )
                        # batched read-back of this head's shards
                        attn_all[h] = p3a.tile([128, W * TC], BF16,
                                               tag=f"attn{h}",
                                               name=f"attn{h}")
                        nc.sync.dma_start(
                            attn_all[h][:].rearrange(
                                "p (i t) -> p i t", i=W, t=TC),
                            a2a_out[h][:, :, :].rearrange("i p t -> p i t"),
                        )

                # ---------------- phase 3: output projection (bf16) --------
                # attn tile for kc = i*HL + h is attn_all[h][:, i*512:...].
                # Even kc (head-0 sourced, available before the second A2A)
                # accumulates first, covering the second A2A's latency.
                with tc.tile_pool(name="ps3", bufs=1, space="PSUM") as ps3:
                    for og in range(2):
                        po3s = {}
                        for oc in (2 * og, 2 * og + 1):
                            for tt in range(TL // 128):
                                po3 = ps3.tile([128, 512], F32,
                                               tag=f"po3_{oc % 2}_{tt}",
                                               name=f"po3_{oc}_{tt}")
                                po3s[(oc, tt)] = po3
                                for i in range(W):
                                    kc = i * HL
                                    nc.tensor.matmul(
                                        po3[:],
                                        attn_all[0][:, i * TC + tt * 128:
                                                    i * TC + (tt + 1) * 128],
                                        wp[:, kc * C + oc * 512:
                                           kc * C + (oc + 1) * 512],
                                        start=(i == 0), stop=False)
                        for oc in (2 * og, 2 * og + 1):
                            for tt in range(TL // 128):
                                po3 = po3s[(oc, tt)]
                                for i in range(W):
                                    kc = i * HL + 1
                                    nc.tensor.matmul(
                                        po3[:],
                                        attn_all[1][:, i * TC + tt * 128:
                                                    i * TC + (tt + 1) * 128],
                                        wp[:, kc * C + oc * 512:
                                           kc * C + (oc + 1) * 512],
                                        start=False, stop=(i == W - 1))
                                ob = p3o.tile([128, 512], F32, tag="ob")
                                nc.scalar.copy(ob[:], po3[:])
                                nc.sync.dma_start(
                                    out_d.ap()[tt * 128:(tt + 1) * 128,
                                               oc * 512:(oc + 1) * 512],
                                    ob[:])

    nc.compile()
    return nc


def _maybe_install_trace_hook():
    try:
        import antenv
        from trn_agent_boot.trn_boot import _ntff_profile_via_ctypes
        hook = _ntff_profile_via_ctypes("/opt/axon/libaxon_pjrt.so")
        mod = types.ModuleType("antenv.axon_hooks")
        mod.get_axon_ntff_profile_hook = lambda: hook
        mod.set_axon_ntff_profile_hook = lambda h: None
        sys.modules["antenv.axon_hooks"] = mod
        antenv.axon_hooks = mod
        return True
    except Exception:
        return False


def kernel(x, w_attn, w_proj):
    x = np.ascontiguousarray(x, dtype=np.float32)
    w_attn = np.ascontiguousarray(w_attn, dtype=np.float32)
    w_proj = np.ascontiguousarray(w_proj, dtype=np.float32)

    if "nc" not in _cache:
        _cache["nc"] = _build()
    nc = _cache["nc"]

    # pre-tile into SBUF layouts (bf16) for max-size DMA descriptors
    xTt = np.ascontiguousarray(
        x.reshape(NC, TC, KT, 128).transpose(0, 3, 2, 1)
        .reshape(NC * 128, KT * TC)).astype(NP_BF16)
    wpTt = np.ascontiguousarray(
        w_proj.T.reshape(KT, 128, C).transpose(1, 0, 2)
        .reshape(128, KT * C)).astype(NP_BF16)
    in_maps = []
    for c in range(W):
        r0 = CL * c
        wqk = np.concatenate(
            [w_attn[r0:r0 + CL], w_attn[C + r0:C + r0 + CL]], axis=0)
        wqkTt = np.ascontiguousarray(
            wqk.T.reshape(KT, 128, 512).transpose(1, 0, 2)
            .reshape(128, KT * 512)).astype(NP_BF16)
        wvTt = np.ascontiguousarray(
            w_attn[2 * C + r0:2 * C + r0 + CL].T
            .reshape(KT, 128, CL).transpose(1, 0, 2)
            .reshape(128, KT * CL)).astype(NP_BF16)
        in_maps.append({"xTt": xTt, "wqkTt": wqkTt, "wvTt": wvTt,
                        "wpTt": wpTt})

    trace = TRACE and _maybe_install_trace_hook()
    res = run_bass_kernel_spmd(nc, in_maps, list(range(W)), trace=trace)
    LAST_RESULT["exec_time_ns"] = res.exec_time_ns

    return np.concatenate([res.results[c]["out"] for c in range(W)], axis=0)


# revision 7
# speedup vs baseline: 1.0670x; 1.0670x over previous
"""Causal self-attention (T=4096, C=2048, 16 heads) on 8 TRN2 NeuronCores.

Sharding: tensor-parallel over heads (2 heads/core) for QKV + attention,
then per-head AllToAlls redistribute the attention output to
token-parallel (512 tokens/core) for the output projection; each core
computes full output rows for its token slice and the host concatenates.

All matmuls run bf16 (host converts, halving DMA bytes; PSUM stays
fp32). Scores are computed transposed (keys on partitions, queries
free) so P@V needs no transposes; causal masking is a bf16 multiply
with precomputed diagonal masks and upper-triangle blocks are skipped.

Performance structure:
- Every bulk HBM tensor is PRE-TILED on the host into the exact SBUF
  layout, so each load is one dma_start with 8-32KB descriptors (the
  DMA rings charge ~155ns per descriptor regardless of size, so 1KB
  descriptors cap loads at ~105GB/s while 16KB descriptors saturate).
- Exp runs on [128,1024] pairs of score tiles; on the last (diagonal)
  pair of each chunk only the causally-live [128,2,256] sub-block is
  computed.
- Softmax denominators: DVE accumulates exp sums into a [128,1024]
  running tile, folds once per chunk, and a single ones-vector matmul
  per chunk reduces over partitions (per-k-tile ones-matmuls would
  burn ~60us of tensor time).
- Normalization happens on the SOURCE side of the AllToAll (reciprocal
  + partition-broadcast + DVE multiply, off the tensor path), so phase
  3 is pure DMA + matmul. The gpsimd partition-broadcast library is
  warmed up at t=0: the first gpsimd op of a new type triggers a ~14us
  LIBRARY_RELOAD which otherwise lands mid-pipeline.
- qT/kT PSUM->SBUF copies run on the scalar engine (idle in phase 1)
  to keep the DVE free for phase-2 exp-sum accumulation.
"""
import sys
import types

sys.path.insert(0, "/opt/trn_rl_repo")

import ml_dtypes
import numpy as np

from concourse import bacc, tile
import concourse.mybir as mybir
from concourse.bass_utils import run_bass_kernel_spmd

F32 = mybir.dt.float32
BF16 = mybir.dt.bfloat16
NP_BF16 = np.dtype(ml_dtypes.bfloat16)

T, C = 4096, 2048
H, D = 16, 128
W = 8                  # cores
HL = H // W            # heads per core (2)
CL = HL * D            # local attention-output columns (256)
KT = C // 128          # contraction tiles (16)
TC = 512               # token chunk
NC = T // TC           # 8
TL = T // W            # tokens per core for the projection (512)
SCALE = float(1.0 / np.sqrt(D))

TRACE = False          # test harness sets kernel.TRACE = True for profiling
LAST_RESULT = {}       # test harness reads exec_time_ns from here

_cache = {}


def _build():
    nc = bacc.Bacc("TRN2", target_bir_lowering=False, debug=False, num_devices=W)
    # host pre-tiles everything into [partition, free] SBUF layout
    xTt_d = nc.dram_tensor("xTt", [NC * 128, KT * TC], BF16,
                           kind="ExternalInput")
    wqkTt_d = nc.dram_tensor("wqkTt", [128, KT * 512], BF16,
                             kind="ExternalInput")
    wvTt_d = nc.dram_tensor("wvTt", [128, KT * CL], BF16,
                            kind="ExternalInput")
    wpTt_d = nc.dram_tensor("wpTt", [128, KT * C], BF16,
                            kind="ExternalInput")
    out_d = nc.dram_tensor("out", [TL, C], F32, kind="ExternalOutput")

    with tile.TileContext(nc) as tc:
        with tc.tile_pool(name="res", bufs=1) as res, \
             tc.tile_pool(name="dram", bufs=1, space="DRAM") as dram:
            # per-head A2A buffers (bf16): shard j = my token chunk j.
            a2a_in = [dram.tile([W, 128, TC], BF16, tag=f"a2a_in{h}",
                                name=f"a2a_in{h}") for h in range(HL)]
            a2a_out = [dram.tile([W, 128, TC], BF16, tag=f"a2a_out{h}",
                                 name=f"a2a_out{h}") for h in range(HL)]

            # resident q/k (transposed, [d, t]) and V ([s, d]), all bf16
            qT = [res.tile([128, T], BF16, tag=f"qT{h}", name=f"qT{h}")
                  for h in range(HL)]
            kT = [res.tile([128, T], BF16, tag=f"kT{h}", name=f"kT{h}")
                  for h in range(HL)]
            V = [res.tile([128, CL], BF16, tag=f"V{i}", name=f"V{i}")
                 for i in range(T // 128)]

            ones32 = res.tile([128, 1], F32, tag="ones32")
            nc.gpsimd.memset(ones32[:], 1.0)
            ones = res.tile([128, 1], BF16, tag="ones")
            nc.vector.tensor_copy(ones[:], ones32[:])

            # paired diagonal causal masks (keep where t >= s within the
            # tile): [mask_dk0 | mask_dk0+1] as one [128,1024] tile
            masks = []
            for dk0 in (0, 2):
                m32 = res.tile([128, 1024], F32, tag=f"m32_{dk0}",
                               name=f"m32_{dk0}")
                nc.gpsimd.memset(m32[:], 1.0)
                mb = res.tile([128, 1024], BF16, tag=f"mask{dk0}",
                              name=f"mask{dk0}")
                nc.vector.tensor_copy(mb[:], m32[:])
                for half in range(2):
                    nc.gpsimd.affine_select(
                        out=mb[:, half * 512:(half + 1) * 512],
                        in_=mb[:, half * 512:(half + 1) * 512],
                        compare_op=mybir.AluOpType.is_ge,
                        fill=0.0,
                        base=-128 * (dk0 + half),
                        channel_multiplier=-1,
                        pattern=[[1, 512]],
                    )
                masks.append(mb)

            # warm up the gpsimd partition-broadcast DSP library now, while
            # the initial DMAs stream: the first op of a type triggers a
            # ~14us LIBRARY_RELOAD that must not land mid-attention
            warm = res.tile([128, 1], F32, tag="warm")
            nc.gpsimd.partition_broadcast(warm[:], ones32[0:1, :])

            # ---------------- phase 1: QKV projection (bf16) ----------------
            with tc.tile_pool(name="wpool", bufs=1) as wpool, \
                 tc.tile_pool(name="xpool", bufs=2) as xpool, \
                 tc.tile_pool(name="ps1", bufs=3, space="PSUM") as ps1:
                wqk = wpool.tile([128, KT * 512], BF16, tag="wqk", name="wqk")
                nc.sync.dma_start(wqk[:], wqkTt_d.ap())

                def load_x_chunk(j):
                    xt = xpool.tile([128, KT * TC], BF16, tag="x",
                                    name=f"x{j}")
                    nc.sync.dma_start(
                        xt[:], xTt_d.ap()[j * 128:(j + 1) * 128, :])
                    return xt

                xt0 = load_x_chunk(0)
                wv = wpool.tile([128, KT * CL], BF16, tag="wv", name="wv")
                nc.sync.dma_start(wv[:], wvTt_d.ap())

                for j in range(NC):
                    xt = xt0 if j == 0 else load_x_chunk(j)
                    # qT/kT for both heads: out[d, t] accumulated over c
                    for m in range(4):
                        pq = ps1.tile([128, TC], F32, tag="pqk")
                        for k in range(KT):
                            nc.tensor.matmul(
                                pq[:],
                                wqk[:, k * 512 + m * 128:
                                    k * 512 + (m + 1) * 128],
                                xt[:, k * TC:(k + 1) * TC],
                                start=(k == 0), stop=(k == KT - 1))
                        dest = qT[m] if m < HL else kT[m - HL]
                        nc.vector.tensor_copy(
                            dest[:, j * TC:(j + 1) * TC], pq[:])
                    # V: out[t, d] accumulated over c
                    for tt in range(TC // 128):
                        pv = ps1.tile([128, CL], F32, tag="pv")
                        for k in range(KT):
                            nc.tensor.matmul(
                                pv[:],
                                xt[:, k * TC + tt * 128:
                                   k * TC + (tt + 1) * 128],
                                wv[:, k * CL:(k + 1) * CL],
                                start=(k == 0), stop=(k == KT - 1))
                        nc.scalar.copy(V[j * (TC // 128) + tt][:], pv[:])

            # ---------------- phases 2+3 pools ----------------
            with tc.tile_pool(name="ph2", bufs=3) as p2, \
                 tc.tile_pool(name="a2s", bufs=3) as a2s, \
                 tc.tile_pool(name="p3a", bufs=1) as p3a, \
                 tc.tile_pool(name="p2n", bufs=2) as p2n, \
                 tc.tile_pool(name="p3w", bufs=1) as p3w, \
                 tc.tile_pool(name="p3o", bufs=2) as p3o:
                # prefetch the projection weight during phase 2 (one
                # dispatch, 32KB descriptors)
                wp = p3w.tile([128, KT * C], BF16, tag="wp", name="wp")
                nc.sync.dma_start(
                    wp[:].rearrange("p (h x) -> p h x", h=2),
                    wpTt_d.ap().rearrange("p (h x) -> p h x", h=2),
                )

                attn_all = [None] * HL

                # ---------------- phase 2: attention (bf16) ----------------
                with tc.tile_pool(name="ps2s", bufs=2, space="PSUM") as ps2s, \
                     tc.tile_pool(name="ps2o", bufs=2, space="PSUM") as ps2o, \
                     tc.tile_pool(name="ps2d", bufs=1, space="PSUM") as ps2d:
                    pending = None
                    for h in range(HL):
                        for j in range(NC):
                            npairs = (j + 1) * 2
                            po = ps2o.tile([128, TC], F32, tag="po")
                            esum = p2n.tile([128, 1024], BF16, tag="esum")
                            for p in range(npairs):
                                k0, k1 = 2 * p, 2 * p + 1
                                dk0 = k0 - 4 * j
                                # causally-live column offset for this pair
                                off = 256 if dk0 == 2 else 0
                                ps = ps2s.tile([128, 1024], F32, tag="ps")
                                nc.tensor.matmul(
                                    ps[:, off:512],
                                    kT[h][:, k0 * 128:(k0 + 1) * 128],
                                    qT[h][:, j * TC + off:(j + 1) * TC],
                                    start=True, stop=True)
                                nc.tensor.matmul(
                                    ps[:, 512 + off:1024],
                                    kT[h][:, k1 * 128:(k1 + 1) * 128],
                                    qT[h][:, j * TC + off:(j + 1) * TC],
                                    start=True, stop=True)
                                if p == 1 and pending is not None:
                                    # previous chunk's denominator + norm,
                                    # deferred behind a pair of scores
                                    pending()
                                    pending = None
                                e2 = p2.tile([128, 1024], BF16, tag="e")
                                if off:
                                    nc.scalar.activation(
                                        e2[:].rearrange(
                                            "p (g q) -> p g q",
                                            g=2)[:, :, off:],
                                        ps[:].rearrange(
                                            "p (g q) -> p g q",
                                            g=2)[:, :, off:],
                                        mybir.ActivationFunctionType.Exp,
                                        scale=SCALE)
                                elif p == 0:
                                    # split the first pair's exp so the
                                    # chunk's first PV only waits on half a
                                    # tile (kills the per-chunk pipeline
                                    # refill bubble on the tensor engine)
                                    nc.scalar.activation(
                                        e2[:, 0:512], ps[:, 0:512],
                                        mybir.ActivationFunctionType.Exp,
                                        scale=SCALE)
                                    nc.scalar.activation(
                                        e2[:, 512:1024], ps[:, 512:1024],
                                        mybir.ActivationFunctionType.Exp,
                                        scale=SCALE)
                                else:
                                    nc.scalar.activation(
                                        e2[:], ps[:],
                                        mybir.ActivationFunctionType.Exp,
                                        scale=SCALE)
                                if dk0 >= 0:
                                    # diagonal pair: zero out s > t entries
                                    mk = masks[dk0 // 2]
                                    if off:
                                        sl = e2[:].rearrange(
                                            "p (g q) -> p g q",
                                            g=2)[:, :, off:]
                                        nc.vector.tensor_mul(
                                            sl, sl,
                                            mk[:].rearrange(
                                                "p (g q) -> p g q",
                                                g=2)[:, :, off:])
                                    else:
                                        nc.vector.tensor_mul(
                                            e2[:], e2[:], mk[:])
                                if p == 0:
                                    nc.vector.tensor_copy(esum[:], e2[:])
                                elif off:
                                    se = esum[:].rearrange(
                                        "p (g q) -> p g q", g=2)[:, :, off:]
                                    nc.vector.tensor_add(
                                        se, se,
                                        e2[:].rearrange(
                                            "p (g q) -> p g q",
                                            g=2)[:, :, off:])
                                else:
                                    nc.vector.tensor_add(
                                        esum[:], esum[:], e2[:])
                                nc.tensor.matmul(
                                    po[:, off:512],
                                    V[k0][:, h * 128:(h + 1) * 128],
                                    e2[:, off:512],
                                    start=(p == 0), stop=False)
                                nc.tensor.matmul(
                                    po[:, off:512],
                                    V[k1][:, h * 128:(h + 1) * 128],
                                    e2[:, 512 + off:1024],
                                    start=False, stop=(p == npairs - 1))
                            # fold the two esum halves (DVE, off tensor path)
                            nc.vector.tensor_add(
                                esum[:, 0:512], esum[:, 0:512],
                                esum[:, 512:1024])

                            def make_norm(h=h, j=j, po=po, esum=esum):
                                def norm():
                                    pd = ps2d.tile([1, TC], F32, tag="pd")
                                    nc.tensor.matmul(pd[:], ones[:],
                                                     esum[:, 0:512],
                                                     start=True, stop=True)
                                    rec = p2n.tile([1, TC], F32, tag="rec")
                                    nc.vector.reciprocal(rec[:], pd[:])
                                    r128 = p2n.tile([128, TC], F32,
                                                    tag="r128")
                                    nc.gpsimd.partition_broadcast(
                                        r128[:], rec[:])
                                    att = a2s.tile([128, TC], BF16,
                                                   tag="att")
                                    nc.vector.tensor_mul(att[:], po[:],
                                                         r128[:])
                                    nc.sync.dma_start(a2a_in[h][j, :, :],
                                                      att[:])
                                return norm

                            pending = make_norm()
                        # flush the last chunk's normalize, then fire this
                        # head's A2A; head 0's collective overlaps head 1
                        pending()
                        pending = None
                        nc.gpsimd.collective_compute(
                            "AllToAll",
                            mybir.AluOpType.bypass,
                            ins=[a2a_in[h].opt()],
                            outs=[a2a_out[h].opt()],
                            replica_groups=[list(range(W))],
                        )
                        # batched read-back of this head's shards
                        attn_all[h] = p3a.tile([128, W * TC], BF16,
                                               tag=f"attn{h}",
                                               name=f"attn{h}")
                        nc.sync.dma_start(
                            attn_all[h][:].rearrange(
                                "p (i t) -> p i t", i=W, t=TC),
                            a2a_out[h][:, :, :].rearrange("i p t -> p i t"),
                        )

                # ---------------- phase 3: output projection (bf16) --------
                # attn tile for kc = i*HL + h is attn_all[h][:, i*512:...].
                # Even kc (head-0 sourced, available before the second A2A)
                # accumulates first, covering the second A2A's latency.
                with tc.tile_pool(name="ps3", bufs=1, space="PSUM") as ps3:
                    for og in range(2):
                        po3s = {}
                        for oc in (2 * og, 2 * og + 1):
                            for tt in range(TL // 128):
                                po3 = ps3.tile([128, 512], F32,
                                               tag=f"po3_{oc % 2}_{tt}",
                                               name=f"po3_{oc}_{tt}")
                                po3s[(oc, tt)] = po3
                                for i in range(W):
                                    kc = i * HL
                                    nc.tensor.matmul(
                                        po3[:],
                                        attn_all[0][:, i * TC + tt * 128:
                                                    i * TC + (tt + 1) * 128],
                                        wp[:, kc * C + oc * 512:
                                           kc * C + (oc + 1) * 512],
                                        start=(i == 0), stop=False)
                        for oc in (2 * og, 2 * og + 1):
                            for tt in range(TL // 128):
                                po3 = po3s[(oc, tt)]
                                for i in range(W):
                                    kc = i * HL + 1
                                    nc.tensor.matmul(
                                        po3[:],
                                        attn_all[1][:, i * TC + tt * 128:
                                                    i * TC + (tt + 1) * 128],
                                        wp[:, kc * C + oc * 512:
                                           kc * C + (oc + 1) * 512],
                                        start=False, stop=(i == W - 1))
                                ob = p3o.tile([128, 512], F32, tag="ob")
                                nc.scalar.copy(ob[:], po3[:])
                                nc.sync.dma_start(
                                    out_d.ap()[tt * 128:(tt + 1) * 128,
                                               oc * 512:(oc + 1) * 512],
                                    ob[:])

    nc.compile()
    return nc


def _maybe_install_trace_hook():
    try:
        import antenv
        from trn_agent_boot.trn_boot import _ntff_profile_via_ctypes
        hook = _ntff_profile_via_ctypes("/opt/axon/libaxon_pjrt.so")
        mod = types.ModuleType("antenv.axon_hooks")
        mod.get_axon_ntff_profile_hook = lambda: hook
        mod.set_axon_ntff_profile_hook = lambda h: None
        sys.modules["antenv.axon_hooks"] = mod
        antenv.axon_hooks = mod
        return True
    except Exception:
        return False


def kernel(x, w_attn, w_proj):
    x = np.ascontiguousarray(x, dtype=np.float32)
    w_attn = np.ascontiguousarray(w_attn, dtype=np.float32)
    w_proj = np.ascontiguousarray(w_proj, dtype=np.float32)

    if "nc" not in _cache:
        _cache["nc"] = _build()
    nc = _cache["nc"]

    # pre-tile into SBUF layouts (bf16) for max-size DMA descriptors
    xTt = np.ascontiguousarray(
        x.reshape(NC, TC, KT, 128).transpose(0, 3, 2, 1)
        .reshape(NC * 128, KT * TC)).astype(NP_BF16)
    wpTt = np.ascontiguousarray(
        w_proj.T.reshape(KT, 128, C).transpose(1, 0, 2)
        .reshape(128, KT * C)).astype(NP_BF16)
    in_maps = []
    for c in range(W):
        r0 = CL * c
        wqk = np.concatenate(
            [w_attn[r0:r0 + CL], w_attn[C + r0:C + r0 + CL]], axis=0)
        wqkTt = np.ascontiguousarray(
            wqk.T.reshape(KT, 128, 512).transpose(1, 0, 2)
            .reshape(128, KT * 512)).astype(NP_BF16)
        wvTt = np.ascontiguousarray(
            w_attn[2 * C + r0:2 * C + r0 + CL].T
            .reshape(KT, 128, CL).transpose(1, 0, 2)
            .reshape(128, KT * CL)).astype(NP_BF16)
        in_maps.append({"xTt": xTt, "wqkTt": wqkTt, "wvTt": wvTt,
                        "wpTt": wpTt})

    trace = TRACE and _maybe_install_trace_hook()
    res = run_bass_kernel_spmd(nc, in_maps, list(range(W)), trace=trace)
    LAST_RESULT["exec_time_ns"] = res.exec_time_ns

    return np.concatenate([res.results[c]["out"] for c in range(W)], axis=0)


# revision 9
# speedup vs baseline: 1.0789x; 1.0112x over previous
"""Causal self-attention (T=4096, C=2048, 16 heads) on 8 TRN2 NeuronCores.

Sharding: tensor-parallel over heads (2 heads/core) for QKV + attention,
then per-head AllToAlls redistribute the attention output to
token-parallel (512 tokens/core) for the output projection; each core
computes full output rows for its token slice and the host concatenates.

All matmuls run bf16 (host converts, halving DMA bytes; PSUM stays
fp32). Scores are computed transposed (keys on partitions, queries
free) so P@V needs no transposes; causal masking is a bf16 multiply
with precomputed diagonal masks and upper-triangle blocks are skipped.

Performance structure:
- Every bulk HBM tensor is PRE-TILED on the host into the exact SBUF
  layout, so each load is one dma_start with 8-32KB descriptors (the
  DMA rings charge ~155ns per descriptor regardless of size, so 1KB
  descriptors cap loads at ~105GB/s while 16KB descriptors saturate).
- Exp runs on [128,1024] pairs of score tiles; on the last (diagonal)
  pair of each chunk only the causally-live [128,2,256] sub-block is
  computed.
- Softmax denominators: DVE accumulates exp sums into a [128,1024]
  running tile, folds once per chunk, and a single ones-vector matmul
  per chunk reduces over partitions (per-k-tile ones-matmuls would
  burn ~60us of tensor time).
- Normalization happens on the SOURCE side of the AllToAll (reciprocal
  + partition-broadcast + DVE multiply, off the tensor path), so phase
  3 is pure DMA + matmul. The gpsimd partition-broadcast library is
  warmed up at t=0: the first gpsimd op of a new type triggers a ~14us
  LIBRARY_RELOAD which otherwise lands mid-pipeline.
- qT/kT PSUM->SBUF copies run on the scalar engine (idle in phase 1)
  to keep the DVE free for phase-2 exp-sum accumulation.
"""
import sys
import types

sys.path.insert(0, "/opt/trn_rl_repo")

import ml_dtypes
import numpy as np

from concourse import bacc, tile
import concourse.mybir as mybir
from concourse.bass_utils import run_bass_kernel_spmd

F32 = mybir.dt.float32
BF16 = mybir.dt.bfloat16
NP_BF16 = np.dtype(ml_dtypes.bfloat16)

T, C = 4096, 2048
H, D = 16, 128
W = 8                  # cores
HL = H // W            # heads per core (2)
CL = HL * D            # local attention-output columns (256)
KT = C // 128          # contraction tiles (16)
TC = 512               # token chunk
NC = T // TC           # 8
TL = T // W            # tokens per core for the projection (512)
SCALE = float(1.0 / np.sqrt(D))

TRACE = False          # test harness sets kernel.TRACE = True for profiling
LAST_RESULT = {}       # test harness reads exec_time_ns from here

_cache = {}


def _build():
    nc = bacc.Bacc("TRN2", target_bir_lowering=False, debug=False, num_devices=W)
    # host pre-tiles everything into [partition, free] SBUF layout
    xTt_d = nc.dram_tensor("xTt", [NC * 128, KT * TC], BF16,
                           kind="ExternalInput")
    wqkTt_d = nc.dram_tensor("wqkTt", [128, KT * 512], BF16,
                             kind="ExternalInput")
    wvTt_d = nc.dram_tensor("wvTt", [128, KT * CL], BF16,
                            kind="ExternalInput")
    wpTt_d = nc.dram_tensor("wpTt", [128, KT * C], BF16,
                            kind="ExternalInput")
    out_d = nc.dram_tensor("out", [TL, C], F32, kind="ExternalOutput")

    with tile.TileContext(nc) as tc:
        with tc.tile_pool(name="res", bufs=1) as res, \
             tc.tile_pool(name="dram", bufs=1, space="DRAM") as dram:
            # per-head A2A buffers (bf16): shard j = my token chunk j.
            a2a_in = [dram.tile([W, 128, TC], BF16, tag=f"a2a_in{h}",
                                name=f"a2a_in{h}") for h in range(HL)]
            a2a_out = [dram.tile([W, 128, TC], BF16, tag=f"a2a_out{h}",
                                 name=f"a2a_out{h}") for h in range(HL)]

            # resident q/k (transposed, [d, t]) and V ([s, d]), all bf16
            qT = [res.tile([128, T], BF16, tag=f"qT{h}", name=f"qT{h}")
                  for h in range(HL)]
            kT = [res.tile([128, T], BF16, tag=f"kT{h}", name=f"kT{h}")
                  for h in range(HL)]
            V = [res.tile([128, CL], BF16, tag=f"V{i}", name=f"V{i}")
                 for i in range(T // 128)]

            ones32 = res.tile([128, 1], F32, tag="ones32")
            nc.gpsimd.memset(ones32[:], 1.0)
            ones = res.tile([128, 1], BF16, tag="ones")
            nc.vector.tensor_copy(ones[:], ones32[:])

            # paired diagonal causal masks (keep where t >= s within the
            # tile): [mask_dk0 | mask_dk0+1] as one [128,1024] tile
            masks = []
            for dk0 in (0, 2):
                m32 = res.tile([128, 1024], F32, tag=f"m32_{dk0}",
                               name=f"m32_{dk0}")
                nc.gpsimd.memset(m32[:], 1.0)
                mb = res.tile([128, 1024], BF16, tag=f"mask{dk0}",
                              name=f"mask{dk0}")
                nc.vector.tensor_copy(mb[:], m32[:])
                for half in range(2):
                    nc.gpsimd.affine_select(
                        out=mb[:, half * 512:(half + 1) * 512],
                        in_=mb[:, half * 512:(half + 1) * 512],
                        compare_op=mybir.AluOpType.is_ge,
                        fill=0.0,
                        base=-128 * (dk0 + half),
                        channel_multiplier=-1,
                        pattern=[[1, 512]],
                    )
                masks.append(mb)

            # warm up the gpsimd partition-broadcast DSP library now, while
            # the initial DMAs stream: the first op of a type triggers a
            # ~14us LIBRARY_RELOAD that must not land mid-attention
            warm = res.tile([128, 1], F32, tag="warm")
            nc.gpsimd.partition_broadcast(warm[:], ones32[0:1, :])

            # ---------------- phase 1: QKV projection (bf16) ----------------
            with tc.tile_pool(name="wpool", bufs=1) as wpool, \
                 tc.tile_pool(name="xpool", bufs=2) as xpool, \
                 tc.tile_pool(name="ps1", bufs=3, space="PSUM") as ps1:
                wqk = wpool.tile([128, KT * 512], BF16, tag="wqk", name="wqk")
                nc.sync.dma_start(wqk[:], wqkTt_d.ap())

                def load_x_chunk(j):
                    xt = xpool.tile([128, KT * TC], BF16, tag="x",
                                    name=f"x{j}")
                    nc.sync.dma_start(
                        xt[:], xTt_d.ap()[j * 128:(j + 1) * 128, :])
                    return xt

                xt0 = load_x_chunk(0)
                wv = wpool.tile([128, KT * CL], BF16, tag="wv", name="wv")
                nc.sync.dma_start(wv[:], wvTt_d.ap())

                exp_warm = res.tile([1, 1], BF16, tag="exp_warm")
                for j in range(NC):
                    if j == 6:
                        # preload the Exp activation table (1.3us) while
                        # phase 1 still computes; Copy is tableless so the
                        # remaining V copies don't evict it
                        nc.scalar.activation(
                            exp_warm[:], ones32[0:1, :],
                            mybir.ActivationFunctionType.Exp)
                    xt = xt0 if j == 0 else load_x_chunk(j)
                    # qT/kT for both heads: out[d, t] accumulated over c
                    for m in range(4):
                        pq = ps1.tile([128, TC], F32, tag="pqk")
                        for k in range(KT):
                            nc.tensor.matmul(
                                pq[:],
                                wqk[:, k * 512 + m * 128:
                                    k * 512 + (m + 1) * 128],
                                xt[:, k * TC:(k + 1) * TC],
                                start=(k == 0), stop=(k == KT - 1))
                        dest = qT[m] if m < HL else kT[m - HL]
                        nc.vector.tensor_copy(
                            dest[:, j * TC:(j + 1) * TC], pq[:])
                    # V: out[t, d] accumulated over c
                    for tt in range(TC // 128):
                        pv = ps1.tile([128, CL], F32, tag="pv")
                        for k in range(KT):
                            nc.tensor.matmul(
                                pv[:],
                                xt[:, k * TC + tt * 128:
                                   k * TC + (tt + 1) * 128],
                                wv[:, k * CL:(k + 1) * CL],
                                start=(k == 0), stop=(k == KT - 1))
                        nc.scalar.copy(V[j * (TC // 128) + tt][:], pv[:])

            # ---------------- phases 2+3 pools ----------------
            with tc.tile_pool(name="ph2", bufs=3) as p2, \
                 tc.tile_pool(name="a2s", bufs=3) as a2s, \
                 tc.tile_pool(name="p3a", bufs=1) as p3a, \
                 tc.tile_pool(name="p2n", bufs=2) as p2n, \
                 tc.tile_pool(name="p3w", bufs=1) as p3w, \
                 tc.tile_pool(name="p3o", bufs=2) as p3o:
                # prefetch the projection weight during phase 2 (one
                # dispatch, 32KB descriptors)
                wp = p3w.tile([128, KT * C], BF16, tag="wp", name="wp")
                nc.sync.dma_start(
                    wp[:].rearrange("p (h x) -> p h x", h=2),
                    wpTt_d.ap().rearrange("p (h x) -> p h x", h=2),
                )

                attn_all = [None] * HL

                # ---------------- phase 2: attention (bf16) ----------------
                with tc.tile_pool(name="ps2s", bufs=2, space="PSUM") as ps2s, \
                     tc.tile_pool(name="ps2o", bufs=2, space="PSUM") as ps2o, \
                     tc.tile_pool(name="ps2d", bufs=1, space="PSUM") as ps2d:
                    pending = None
                    for h in range(HL):
                        for j in range(NC):
                            npairs = (j + 1) * 2
                            po = ps2o.tile([128, TC], F32, tag="po")
                            esum = p2n.tile([128, 1024], BF16, tag="esum")
                            for p in range(npairs):
                                k0, k1 = 2 * p, 2 * p + 1
                                dk0 = k0 - 4 * j
                                # causally-live column offset for this pair
                                off = 256 if dk0 == 2 else 0
                                ps = ps2s.tile([128, 1024], F32, tag="ps")
                                nc.tensor.matmul(
                                    ps[:, off:512],
                                    kT[h][:, k0 * 128:(k0 + 1) * 128],
                                    qT[h][:, j * TC + off:(j + 1) * TC],
                                    start=True, stop=True)
                                nc.tensor.matmul(
                                    ps[:, 512 + off:1024],
                                    kT[h][:, k1 * 128:(k1 + 1) * 128],
                                    qT[h][:, j * TC + off:(j + 1) * TC],
                                    start=True, stop=True)
                                if p == 1 and pending is not None:
                                    # previous chunk's denominator + norm,
                                    # deferred behind a pair of scores
                                    pending()
                                    pending = None
                                e2 = p2.tile([128, 1024], BF16, tag="e")
                                if off:
                                    nc.scalar.activation(
                                        e2[:].rearrange(
                                            "p (g q) -> p g q",
                                            g=2)[:, :, off:],
                                        ps[:].rearrange(
                                            "p (g q) -> p g q",
                                            g=2)[:, :, off:],
                                        mybir.ActivationFunctionType.Exp,
                                        scale=SCALE)
                                elif p == 0:
                                    # split the first pair's exp so the
                                    # chunk's first PV only waits on half a
                                    # tile (kills the per-chunk pipeline
                                    # refill bubble on the tensor engine)
                                    nc.scalar.activation(
                                        e2[:, 0:512], ps[:, 0:512],
                                        mybir.ActivationFunctionType.Exp,
                                        scale=SCALE)
                                    nc.scalar.activation(
                                        e2[:, 512:1024], ps[:, 512:1024],
                                        mybir.ActivationFunctionType.Exp,
                                        scale=SCALE)
                                else:
                                    nc.scalar.activation(
                                        e2[:], ps[:],
                                        mybir.ActivationFunctionType.Exp,
                                        scale=SCALE)
                                if dk0 >= 0:
                                    # diagonal pair: zero out s > t entries
                                    mk = masks[dk0 // 2]
                                    if off:
                                        sl = e2[:].rearrange(
                                            "p (g q) -> p g q",
                                            g=2)[:, :, off:]
                                        nc.vector.tensor_mul(
                                            sl, sl,
                                            mk[:].rearrange(
                                                "p (g q) -> p g q",
                                                g=2)[:, :, off:])
                                    else:
                                        nc.vector.tensor_mul(
                                            e2[:], e2[:], mk[:])
                                if p == 0:
                                    nc.vector.tensor_copy(esum[:], e2[:])
                                elif off:
                                    se = esum[:].rearrange(
                                        "p (g q) -> p g q", g=2)[:, :, off:]
                                    nc.vector.tensor_add(
                                        se, se,
                                        e2[:].rearrange(
                                            "p (g q) -> p g q",
                                            g=2)[:, :, off:])
                                else:
                                    nc.vector.tensor_add(
                                        esum[:], esum[:], e2[:])
                                nc.tensor.matmul(
                                    po[:, off:512],
                                    V[k0][:, h * 128:(h + 1) * 128],
                                    e2[:, off:512],
                                    start=(p == 0), stop=False)
                                nc.tensor.matmul(
                                    po[:, off:512],
                                    V[k1][:, h * 128:(h + 1) * 128],
                                    e2[:, 512 + off:1024],
                                    start=False, stop=(p == npairs - 1))
                            # fold the two esum halves (DVE, off tensor path)
                            nc.vector.tensor_add(
                                esum[:, 0:512], esum[:, 0:512],
                                esum[:, 512:1024])

                            def make_norm(h=h, j=j, po=po, esum=esum):
                                def norm():
                                    pd = ps2d.tile([1, TC], F32, tag="pd")
                                    nc.tensor.matmul(pd[:], ones[:],
                                                     esum[:, 0:512],
                                                     start=True, stop=True)
                                    rec = p2n.tile([1, TC], F32, tag="rec")
                                    nc.vector.reciprocal(rec[:], pd[:])
                                    r128 = p2n.tile([128, TC], F32,
                                                    tag="r128")
                                    nc.gpsimd.partition_broadcast(
                                        r128[:], rec[:])
                                    att = a2s.tile([128, TC], BF16,
                                                   tag="att")
                                    nc.vector.tensor_mul(att[:], po[:],
                                                         r128[:])
                                    nc.sync.dma_start(a2a_in[h][j, :, :],
                                                      att[:])
                                return norm

                            pending = make_norm()
                        # flush the last chunk's normalize, then fire this
                        # head's A2A; head 0's collective overlaps head 1
                        pending()
                        pending = None
                        nc.gpsimd.collective_compute(
                            "AllToAll",
                            mybir.AluOpType.bypass,
                            ins=[a2a_in[h].opt()],
                            outs=[a2a_out[h].opt()],
                            replica_groups=[list(range(W))],
                        )
                        # batched read-back of this head's shards
                        attn_all[h] = p3a.tile([128, W * TC], BF16,
                                               tag=f"attn{h}",
                                               name=f"attn{h}")
                        nc.sync.dma_start(
                            attn_all[h][:].rearrange(
                                "p (i t) -> p i t", i=W, t=TC),
                            a2a_out[h][:, :, :].rearrange("i p t -> p i t"),
                        )

                # ---------------- phase 3: output projection (bf16) --------
                # attn tile for kc = i*HL + h is attn_all[h][:, i*512:...].
                # Head-0 (even kc) partials for ALL 16 output tiles run
                # before any head-1 work: the first 8 tiles' even partials
                # spill to SBUF so the other 8 can accumulate in PSUM,
                # giving ~35us of tensor work to cover the second A2A.
                def even_mm(po3, oc, tt, i, start, stop):
                    nc.tensor.matmul(
                        po3[:],
                        attn_all[0][:, i * TC + tt * 128:
                                    i * TC + (tt + 1) * 128],
                        wp[:, (i * HL) * C + oc * 512:
                           (i * HL) * C + (oc + 1) * 512],
                        start=start, stop=stop)

                def odd_mm(po3, oc, tt, i, start, stop):
                    nc.tensor.matmul(
                        po3[:],
                        attn_all[1][:, i * TC + tt * 128:
                                    i * TC + (tt + 1) * 128],
                        wp[:, (i * HL + 1) * C + oc * 512:
                           (i * HL + 1) * C + (oc + 1) * 512],
                        start=start, stop=stop)

                with tc.tile_pool(name="ps3", bufs=1, space="PSUM") as ps3, \
                     tc.tile_pool(name="p3s", bufs=1) as p3s:
                    # group 0 (oc 0,1) even partials -> spill to SBUF
                    spills = {}
                    for oc in (0, 1):
                        for tt in range(4):
                            po3 = ps3.tile([128, 512], F32,
                                           tag=f"po3_{oc}_{tt}",
                                           name=f"po3e_{oc}_{tt}")
                            for i in range(W):
                                even_mm(po3, oc, tt, i, i == 0, i == W - 1)
                            sp = p3s.tile([128, 512], F32,
                                          tag=f"sp_{oc}_{tt}",
                                          name=f"sp_{oc}_{tt}")
                            nc.vector.tensor_copy(sp[:], po3[:])
                            spills[(oc, tt)] = sp
                    # group 1 (oc 2,3) evens stay in PSUM, then odds
                    po3s = {}
                    for oc in (2, 3):
                        for tt in range(4):
                            po3 = ps3.tile([128, 512], F32,
                                           tag=f"po3_{oc % 2}_{tt}",
                                           name=f"po3f_{oc}_{tt}")
                            po3s[(oc, tt)] = po3
                            for i in range(W):
                                even_mm(po3, oc, tt, i, i == 0, False)
                    for oc in (2, 3):
                        for tt in range(4):
                            po3 = po3s[(oc, tt)]
                            for i in range(W):
                                odd_mm(po3, oc, tt, i, False, i == W - 1)
                    for tt in range(4):
                        ob = p3o.tile([128, 1024], F32, tag="ob")
                        nc.scalar.copy(ob[:, 0:512], po3s[(2, tt)][:])
                        nc.scalar.copy(ob[:, 512:1024], po3s[(3, tt)][:])
                        nc.sync.dma_start(
                            out_d.ap()[tt * 128:(tt + 1) * 128, 1024:2048],
                            ob[:])
                    # group 0 odds into fresh PSUM, then add the spills
                    for tt in range(4):
                        po3a = ps3.tile([128, 512], F32, tag=f"po3_0_{tt}",
                                        name=f"po3o_0_{tt}")
                        po3b = ps3.tile([128, 512], F32, tag=f"po3_1_{tt}",
                                        name=f"po3o_1_{tt}")
                        for i in range(W):
                            odd_mm(po3a, 0, tt, i, i == 0, i == W - 1)
                        for i in range(W):
                            odd_mm(po3b, 1, tt, i, i == 0, i == W - 1)
                        ob = p3o.tile([128, 1024], F32, tag="ob")
                        nc.vector.tensor_add(
                            ob[:, 0:512], spills[(0, tt)][:], po3a[:])
                        nc.vector.tensor_add(
                            ob[:, 512:1024], spills[(1, tt)][:], po3b[:])
                        nc.sync.dma_start(
                            out_d.ap()[tt * 128:(tt + 1) * 128, 0:1024],
                            ob[:])

    nc.compile()
    return nc


def _maybe_install_trace_hook():
    try:
        import antenv
        from trn_agent_boot.trn_boot import _ntff_profile_via_ctypes
        hook = _ntff_profile_via_ctypes("/opt/axon/libaxon_pjrt.so")
        mod = types.ModuleType("antenv.axon_hooks")
        mod.get_axon_ntff_profile_hook = lambda: hook
        mod.set_axon_ntff_profile_hook = lambda h: None
        sys.modules["antenv.axon_hooks"] = mod
        antenv.axon_hooks = mod
        return True
    except Exception:
        return False


def kernel(x, w_attn, w_proj):
    x = np.ascontiguousarray(x, dtype=np.float32)
    w_attn = np.ascontiguousarray(w_attn, dtype=np.float32)
    w_proj = np.ascontiguousarray(w_proj, dtype=np.float32)

    if "nc" not in _cache:
        _cache["nc"] = _build()
    nc = _cache["nc"]

    # pre-tile into SBUF layouts (bf16) for max-size DMA descriptors
    xTt = np.ascontiguousarray(
        x.reshape(NC, TC, KT, 128).transpose(0, 3, 2, 1)
        .reshape(NC * 128, KT * TC)).astype(NP_BF16)
    wpTt = np.ascontiguousarray(
        w_proj.T.reshape(KT, 128, C).transpose(1, 0, 2)
        .reshape(128, KT * C)).astype(NP_BF16)
    in_maps = []
    for c in range(W):
        r0 = CL * c
        wqk = np.concatenate(
            [w_attn[r0:r0 + CL], w_attn[C + r0:C + r0 + CL]], axis=0)
        wqkTt = np.ascontiguousarray(
            wqk.T.reshape(KT, 128, 512).transpose(1, 0, 2)
            .reshape(128, KT * 512)).astype(NP_BF16)
        wvTt = np.ascontiguousarray(
            w_attn[2 * C + r0:2 * C + r0 + CL].T
            .reshape(KT, 128, CL).transpose(1, 0, 2)
            .reshape(128, KT * CL)).astype(NP_BF16)
        in_maps.append({"xTt": xTt, "wqkTt": wqkTt, "wvTt": wvTt,
                        "wpTt": wpTt})

    trace = TRACE and _maybe_install_trace_hook()
    res = run_bass_kernel_spmd(nc, in_maps, list(range(W)), trace=trace)
    LAST_RESULT["exec_time_ns"] = res.exec_time_ns

    return np.concatenate([res.results[c]["out"] for c in range(W)], axis=0)


# revision 11
# speedup vs baseline: 1.0802x; 1.0012x over previous
"""Causal self-attention (T=4096, C=2048, 16 heads) on 8 TRN2 NeuronCores.

Sharding: tensor-parallel over heads (2 heads/core) for QKV + attention,
then per-head AllToAlls redistribute the attention output to
token-parallel (512 tokens/core) for the output projection; each core
computes full output rows for its token slice and the host concatenates.

All matmuls run bf16 (host converts, halving DMA bytes; PSUM stays
fp32). Scores are computed transposed (keys on partitions, queries
free) so P@V needs no transposes; causal masking is a bf16 multiply
with precomputed diagonal masks and upper-triangle blocks are skipped.

Performance structure:
- Every bulk HBM tensor is PRE-TILED on the host into the exact SBUF
  layout, so each load is one dma_start with 8-32KB descriptors (the
  DMA rings charge ~155ns per descriptor regardless of size, so 1KB
  descriptors cap loads at ~105GB/s while 16KB descriptors saturate).
- Exp runs on [128,1024] pairs of score tiles; on the last (diagonal)
  pair of each chunk only the causally-live [128,2,256] sub-block is
  computed.
- Softmax denominators: DVE accumulates exp sums into a [128,1024]
  running tile, folds once per chunk, and a single ones-vector matmul
  per chunk reduces over partitions (per-k-tile ones-matmuls would
  burn ~60us of tensor time).
- Normalization happens on the SOURCE side of the AllToAll (reciprocal
  + partition-broadcast + DVE multiply, off the tensor path), so phase
  3 is pure DMA + matmul. The gpsimd partition-broadcast library is
  warmed up at t=0: the first gpsimd op of a new type triggers a ~14us
  LIBRARY_RELOAD which otherwise lands mid-pipeline.
- qT/kT PSUM->SBUF copies run on the scalar engine (idle in phase 1)
  to keep the DVE free for phase-2 exp-sum accumulation.
"""
import sys
import types

sys.path.insert(0, "/opt/trn_rl_repo")

import ml_dtypes
import numpy as np

from concourse import bacc, tile
import concourse.mybir as mybir
from concourse.bass_utils import run_bass_kernel_spmd

F32 = mybir.dt.float32
BF16 = mybir.dt.bfloat16
NP_BF16 = np.dtype(ml_dtypes.bfloat16)

T, C = 4096, 2048
H, D = 16, 128
W = 8                  # cores
HL = H // W            # heads per core (2)
CL = HL * D            # local attention-output columns (256)
KT = C // 128          # contraction tiles (16)
TC = 512               # token chunk
NC = T // TC           # 8
TL = T // W            # tokens per core for the projection (512)
SCALE = float(1.0 / np.sqrt(D))

TRACE = False          # test harness sets kernel.TRACE = True for profiling
LAST_RESULT = {}       # test harness reads exec_time_ns from here

_cache = {}


def _build():
    nc = bacc.Bacc("TRN2", target_bir_lowering=False, debug=False, num_devices=W)
    # host pre-tiles everything into [partition, free] SBUF layout
    xTt_d = nc.dram_tensor("xTt", [NC * 128, KT * TC], BF16,
                           kind="ExternalInput")
    wqkTt_d = nc.dram_tensor("wqkTt", [128, KT * 512], BF16,
                             kind="ExternalInput")
    wvTt_d = nc.dram_tensor("wvTt", [128, KT * CL], BF16,
                            kind="ExternalInput")
    wpTt_d = nc.dram_tensor("wpTt", [128, KT * C], BF16,
                            kind="ExternalInput")
    out_d = nc.dram_tensor("out", [TL, C], BF16, kind="ExternalOutput")

    with tile.TileContext(nc) as tc:
        with tc.tile_pool(name="res", bufs=1) as res, \
             tc.tile_pool(name="dram", bufs=1, space="DRAM") as dram:
            # per-head A2A buffers (bf16): shard j = my token chunk j.
            a2a_in = [dram.tile([W, 128, TC], BF16, tag=f"a2a_in{h}",
                                name=f"a2a_in{h}") for h in range(HL)]
            a2a_out = [dram.tile([W, 128, TC], BF16, tag=f"a2a_out{h}",
                                 name=f"a2a_out{h}") for h in range(HL)]

            # resident q/k (transposed, [d, t]) and V ([s, d]), all bf16
            qT = [res.tile([128, T], BF16, tag=f"qT{h}", name=f"qT{h}")
                  for h in range(HL)]
            kT = [res.tile([128, T], BF16, tag=f"kT{h}", name=f"kT{h}")
                  for h in range(HL)]
            V = [res.tile([128, CL], BF16, tag=f"V{i}", name=f"V{i}")
                 for i in range(T // 128)]

            ones32 = res.tile([128, 1], F32, tag="ones32")
            nc.gpsimd.memset(ones32[:], 1.0)
            ones = res.tile([128, 1], BF16, tag="ones")
            nc.vector.tensor_copy(ones[:], ones32[:])

            # paired diagonal causal masks (keep where t >= s within the
            # tile): [mask_dk0 | mask_dk0+1] as one [128,1024] tile
            masks = []
            for dk0 in (0, 2):
                m32 = res.tile([128, 1024], F32, tag=f"m32_{dk0}",
                               name=f"m32_{dk0}")
                nc.gpsimd.memset(m32[:], 1.0)
                mb = res.tile([128, 1024], BF16, tag=f"mask{dk0}",
                              name=f"mask{dk0}")
                nc.vector.tensor_copy(mb[:], m32[:])
                for half in range(2):
                    nc.gpsimd.affine_select(
                        out=mb[:, half * 512:(half + 1) * 512],
                        in_=mb[:, half * 512:(half + 1) * 512],
                        compare_op=mybir.AluOpType.is_ge,
                        fill=0.0,
                        base=-128 * (dk0 + half),
                        channel_multiplier=-1,
                        pattern=[[1, 512]],
                    )
                masks.append(mb)

            # warm up the gpsimd partition-broadcast DSP library now, while
            # the initial DMAs stream: the first op of a type triggers a
            # ~14us LIBRARY_RELOAD that must not land mid-attention
            warm = res.tile([128, 1], F32, tag="warm")
            nc.gpsimd.partition_broadcast(warm[:], ones32[0:1, :])

            # ---------------- phase 1: QKV projection (bf16) ----------------
            with tc.tile_pool(name="wpool", bufs=1) as wpool, \
                 tc.tile_pool(name="xpool", bufs=2) as xpool, \
                 tc.tile_pool(name="ps1", bufs=3, space="PSUM") as ps1:
                wqk = wpool.tile([128, KT * 512], BF16, tag="wqk", name="wqk")
                nc.sync.dma_start(wqk[:], wqkTt_d.ap())

                def load_x_chunk(j):
                    xt = xpool.tile([128, KT * TC], BF16, tag="x",
                                    name=f"x{j}")
                    nc.sync.dma_start(
                        xt[:], xTt_d.ap()[j * 128:(j + 1) * 128, :])
                    return xt

                xt0 = load_x_chunk(0)
                wv = wpool.tile([128, KT * CL], BF16, tag="wv", name="wv")
                nc.sync.dma_start(wv[:], wvTt_d.ap())

                exp_warm = res.tile([1, 1], BF16, tag="exp_warm")
                for j in range(NC):
                    if j == 6:
                        # preload the Exp activation table (1.3us) while
                        # phase 1 still computes; Copy is tableless so the
                        # remaining V copies don't evict it
                        nc.scalar.activation(
                            exp_warm[:], ones32[0:1, :],
                            mybir.ActivationFunctionType.Exp)
                    xt = xt0 if j == 0 else load_x_chunk(j)
                    # qT/kT for both heads: out[d, t] accumulated over c
                    for m in range(4):
                        pq = ps1.tile([128, TC], F32, tag="pqk")
                        for k in range(KT):
                            nc.tensor.matmul(
                                pq[:],
                                wqk[:, k * 512 + m * 128:
                                    k * 512 + (m + 1) * 128],
                                xt[:, k * TC:(k + 1) * TC],
                                start=(k == 0), stop=(k == KT - 1))
                        dest = qT[m] if m < HL else kT[m - HL]
                        nc.vector.tensor_copy(
                            dest[:, j * TC:(j + 1) * TC], pq[:])
                    # V: out[t, d] accumulated over c
                    for tt in range(TC // 128):
                        pv = ps1.tile([128, CL], F32, tag="pv")
                        for k in range(KT):
                            nc.tensor.matmul(
                                pv[:],
                                xt[:, k * TC + tt * 128:
                                   k * TC + (tt + 1) * 128],
                                wv[:, k * CL:(k + 1) * CL],
                                start=(k == 0), stop=(k == KT - 1))
                        nc.scalar.copy(V[j * (TC // 128) + tt][:], pv[:])

            # ---------------- phases 2+3 pools ----------------
            with tc.tile_pool(name="ph2", bufs=3) as p2, \
                 tc.tile_pool(name="a2s", bufs=3) as a2s, \
                 tc.tile_pool(name="p3a", bufs=1) as p3a, \
                 tc.tile_pool(name="p2n", bufs=2) as p2n, \
                 tc.tile_pool(name="p3w", bufs=1) as p3w, \
                 tc.tile_pool(name="p3o", bufs=2) as p3o:
                # prefetch the projection weight during phase 2 (one
                # dispatch, 32KB descriptors)
                wp = p3w.tile([128, KT * C], BF16, tag="wp", name="wp")
                nc.sync.dma_start(
                    wp[:].rearrange("p (h x) -> p h x", h=2),
                    wpTt_d.ap().rearrange("p (h x) -> p h x", h=2),
                )

                attn_all = [None] * HL

                # ---------------- phase 2: attention (bf16) ----------------
                with tc.tile_pool(name="ps2s", bufs=2, space="PSUM") as ps2s, \
                     tc.tile_pool(name="ps2o", bufs=2, space="PSUM") as ps2o, \
                     tc.tile_pool(name="ps2d", bufs=1, space="PSUM") as ps2d:
                    pending = None
                    for h in range(HL):
                        for j in range(NC):
                            npairs = (j + 1) * 2
                            po = ps2o.tile([128, TC], F32, tag="po")
                            esum = p2n.tile([128, 1024], BF16, tag="esum")
                            for p in range(npairs):
                                k0, k1 = 2 * p, 2 * p + 1
                                dk0 = k0 - 4 * j
                                # causally-live column offset for this pair
                                off = 256 if dk0 == 2 else 0
                                ps = ps2s.tile([128, 1024], F32, tag="ps")
                                nc.tensor.matmul(
                                    ps[:, off:512],
                                    kT[h][:, k0 * 128:(k0 + 1) * 128],
                                    qT[h][:, j * TC + off:(j + 1) * TC],
                                    start=True, stop=True)
                                nc.tensor.matmul(
                                    ps[:, 512 + off:1024],
                                    kT[h][:, k1 * 128:(k1 + 1) * 128],
                                    qT[h][:, j * TC + off:(j + 1) * TC],
                                    start=True, stop=True)
                                if p == 1 and pending is not None:
                                    # previous chunk's denominator + norm,
                                    # deferred behind a pair of scores
                                    pending()
                                    pending = None
                                e2 = p2.tile([128, 1024], BF16, tag="e")
                                if off:
                                    nc.scalar.activation(
                                        e2[:].rearrange(
                                            "p (g q) -> p g q",
                                            g=2)[:, :, off:],
                                        ps[:].rearrange(
                                            "p (g q) -> p g q",
                                            g=2)[:, :, off:],
                                        mybir.ActivationFunctionType.Exp,
                                        scale=SCALE)
                                elif p == 0:
                                    # split the first pair's exp so the
                                    # chunk's first PV only waits on half a
                                    # tile (kills the per-chunk pipeline
                                    # refill bubble on the tensor engine)
                                    nc.scalar.activation(
                                        e2[:, 0:512], ps[:, 0:512],
                                        mybir.ActivationFunctionType.Exp,
                                        scale=SCALE)
                                    nc.scalar.activation(
                                        e2[:, 512:1024], ps[:, 512:1024],
                                        mybir.ActivationFunctionType.Exp,
                                        scale=SCALE)
                                else:
                                    nc.scalar.activation(
                                        e2[:], ps[:],
                                        mybir.ActivationFunctionType.Exp,
                                        scale=SCALE)
                                if dk0 >= 0:
                                    # diagonal pair: zero out s > t entries
                                    mk = masks[dk0 // 2]
                                    if off:
                                        sl = e2[:].rearrange(
                                            "p (g q) -> p g q",
                                            g=2)[:, :, off:]
                                        nc.vector.tensor_mul(
                                            sl, sl,
                                            mk[:].rearrange(
                                                "p (g q) -> p g q",
                                                g=2)[:, :, off:])
                                    else:
                                        nc.vector.tensor_mul(
                                            e2[:], e2[:], mk[:])
                                if p == 0:
                                    nc.vector.tensor_copy(esum[:], e2[:])
                                elif off:
                                    se = esum[:].rearrange(
                                        "p (g q) -> p g q", g=2)[:, :, off:]
                                    nc.vector.tensor_add(
                                        se, se,
                                        e2[:].rearrange(
                                            "p (g q) -> p g q",
                                            g=2)[:, :, off:])
                                else:
                                    nc.vector.tensor_add(
                                        esum[:], esum[:], e2[:])
                                nc.tensor.matmul(
                                    po[:, off:512],
                                    V[k0][:, h * 128:(h + 1) * 128],
                                    e2[:, off:512],
                                    start=(p == 0), stop=False)
                                nc.tensor.matmul(
                                    po[:, off:512],
                                    V[k1][:, h * 128:(h + 1) * 128],
                                    e2[:, 512 + off:1024],
                                    start=False, stop=(p == npairs - 1))
                            # fold the two esum halves (DVE, off tensor path)
                            nc.vector.tensor_add(
                                esum[:, 0:512], esum[:, 0:512],
                                esum[:, 512:1024])

                            def make_norm(h=h, j=j, po=po, esum=esum):
                                def norm():
                                    pd = ps2d.tile([1, TC], F32, tag="pd")
                                    nc.tensor.matmul(pd[:], ones[:],
                                                     esum[:, 0:512],
                                                     start=True, stop=True)
                                    rec = p2n.tile([1, TC], F32, tag="rec")
                                    nc.vector.reciprocal(rec[:], pd[:])
                                    r128 = p2n.tile([128, TC], F32,
                                                    tag="r128")
                                    nc.gpsimd.partition_broadcast(
                                        r128[:], rec[:])
                                    att = a2s.tile([128, TC], BF16,
                                                   tag="att")
                                    nc.vector.tensor_mul(att[:], po[:],
                                                         r128[:])
                                    nc.sync.dma_start(a2a_in[h][j, :, :],
                                                      att[:])
                                return norm

                            pending = make_norm()
                        # flush the last chunk's normalize, then fire this
                        # head's A2A; head 0's collective overlaps head 1
                        pending()
                        pending = None
                        nc.gpsimd.collective_compute(
                            "AllToAll",
                            mybir.AluOpType.bypass,
                            ins=[a2a_in[h].opt()],
                            outs=[a2a_out[h].opt()],
                            replica_groups=[list(range(W))],
                        )
                        # batched read-back of this head's shards
                        attn_all[h] = p3a.tile([128, W * TC], BF16,
                                               tag=f"attn{h}",
                                               name=f"attn{h}")
                        nc.sync.dma_start(
                            attn_all[h][:].rearrange(
                                "p (i t) -> p i t", i=W, t=TC),
                            a2a_out[h][:, :, :].rearrange("i p t -> p i t"),
                        )

                # ---------------- phase 3: output projection (bf16) --------
                # attn tile for kc = i*HL + h is attn_all[h][:, i*512:...].
                # Head-0 (even kc) partials for ALL 16 output tiles run
                # before any head-1 work: the first 8 tiles' even partials
                # spill to SBUF so the other 8 can accumulate in PSUM,
                # giving ~35us of tensor work to cover the second A2A.
                def even_mm(po3, oc, tt, i, start, stop):
                    nc.tensor.matmul(
                        po3[:],
                        attn_all[0][:, i * TC + tt * 128:
                                    i * TC + (tt + 1) * 128],
                        wp[:, (i * HL) * C + oc * 512:
                           (i * HL) * C + (oc + 1) * 512],
                        start=start, stop=stop)

                def odd_mm(po3, oc, tt, i, start, stop):
                    nc.tensor.matmul(
                        po3[:],
                        attn_all[1][:, i * TC + tt * 128:
                                    i * TC + (tt + 1) * 128],
                        wp[:, (i * HL + 1) * C + oc * 512:
                           (i * HL + 1) * C + (oc + 1) * 512],
                        start=start, stop=stop)

                with tc.tile_pool(name="ps3", bufs=1, space="PSUM") as ps3, \
                     tc.tile_pool(name="p3s", bufs=1) as p3s:
                    # group 0 (oc 0,1) even partials -> spill to SBUF
                    spills = {}
                    for oc in (0, 1):
                        for tt in range(4):
                            po3 = ps3.tile([128, 512], F32,
                                           tag=f"po3_{oc}_{tt}",
                                           name=f"po3e_{oc}_{tt}")
                            for i in range(W):
                                even_mm(po3, oc, tt, i, i == 0, i == W - 1)
                            sp = p3s.tile([128, 512], F32,
                                          tag=f"sp_{oc}_{tt}",
                                          name=f"sp_{oc}_{tt}")
                            nc.vector.tensor_copy(sp[:], po3[:])
                            spills[(oc, tt)] = sp
                    # group 1 (oc 2,3) evens stay in PSUM, then odds
                    po3s = {}
                    for oc in (2, 3):
                        for tt in range(4):
                            po3 = ps3.tile([128, 512], F32,
                                           tag=f"po3_{oc % 2}_{tt}",
                                           name=f"po3f_{oc}_{tt}")
                            po3s[(oc, tt)] = po3
                            for i in range(W):
                                even_mm(po3, oc, tt, i, i == 0, False)
                    for oc in (2, 3):
                        for tt in range(4):
                            po3 = po3s[(oc, tt)]
                            for i in range(W):
                                odd_mm(po3, oc, tt, i, False, i == W - 1)
                    for tt in range(4):
                        ob = p3o.tile([128, 1024], BF16, tag="ob")
                        nc.scalar.copy(ob[:, 0:512], po3s[(2, tt)][:])
                        nc.scalar.copy(ob[:, 512:1024], po3s[(3, tt)][:])
                        nc.sync.dma_start(
                            out_d.ap()[tt * 128:(tt + 1) * 128, 1024:2048],
                            ob[:])
                    # group 0 odds into fresh PSUM, then add the spills
                    for tt in range(4):
                        po3a = ps3.tile([128, 512], F32, tag=f"po3_0_{tt}",
                                        name=f"po3o_0_{tt}")
                        po3b = ps3.tile([128, 512], F32, tag=f"po3_1_{tt}",
                                        name=f"po3o_1_{tt}")
                        for i in range(W):
                            odd_mm(po3a, 0, tt, i, i == 0, i == W - 1)
                        for i in range(W):
                            odd_mm(po3b, 1, tt, i, i == 0, i == W - 1)
                        ob = p3o.tile([128, 1024], BF16, tag="ob")
                        nc.vector.tensor_add(
                            ob[:, 0:512], spills[(0, tt)][:], po3a[:])
                        nc.vector.tensor_add(
                            ob[:, 512:1024], spills[(1, tt)][:], po3b[:])
                        nc.sync.dma_start(
                            out_d.ap()[tt * 128:(tt + 1) * 128, 0:1024],
                            ob[:])

    nc.compile()
    return nc


def _maybe_install_trace_hook():
    try:
        import antenv
        from trn_agent_boot.trn_boot import _ntff_profile_via_ctypes
        hook = _ntff_profile_via_ctypes("/opt/axon/libaxon_pjrt.so")
        mod = types.ModuleType("antenv.axon_hooks")
        mod.get_axon_ntff_profile_hook = lambda: hook
        mod.set_axon_ntff_profile_hook = lambda h: None
        sys.modules["antenv.axon_hooks"] = mod
        antenv.axon_hooks = mod
        return True
    except Exception:
        return False


def kernel(x, w_attn, w_proj):
    x = np.ascontiguousarray(x, dtype=np.float32)
    w_attn = np.ascontiguousarray(w_attn, dtype=np.float32)
    w_proj = np.ascontiguousarray(w_proj, dtype=np.float32)

    if "nc" not in _cache:
        _cache["nc"] = _build()
    nc = _cache["nc"]

    # pre-tile into SBUF layouts (bf16) for max-size DMA descriptors
    xTt = np.ascontiguousarray(
        x.reshape(NC, TC, KT, 128).transpose(0, 3, 2, 1)
        .reshape(NC * 128, KT * TC)).astype(NP_BF16)
    wpTt = np.ascontiguousarray(
        w_proj.T.reshape(KT, 128, C).transpose(1, 0, 2)
        .reshape(128, KT * C)).astype(NP_BF16)
    in_maps = []
    for c in range(W):
        r0 = CL * c
        wqk = np.concatenate(
            [w_attn[r0:r0 + CL], w_attn[C + r0:C + r0 + CL]], axis=0)
        wqkTt = np.ascontiguousarray(
            wqk.T.reshape(KT, 128, 512).transpose(1, 0, 2)
            .reshape(128, KT * 512)).astype(NP_BF16)
        wvTt = np.ascontiguousarray(
            w_attn[2 * C + r0:2 * C + r0 + CL].T
            .reshape(KT, 128, CL).transpose(1, 0, 2)
            .reshape(128, KT * CL)).astype(NP_BF16)
        in_maps.append({"xTt": xTt, "wqkTt": wqkTt, "wvTt": wvTt,
                        "wpTt": wpTt})

    trace = TRACE and _maybe_install_trace_hook()
    res = run_bass_kernel_spmd(nc, in_maps, list(range(W)), trace=trace)
    LAST_RESULT["exec_time_ns"] = res.exec_time_ns

    return np.concatenate([res.results[c]["out"] for c in range(W)],
                          axis=0).astype(np.float32)
